# revision 1
# baseline (speedup 1.0000x reference)
"""CoarsenLattice forward on 8 Trainium2 NeuronCores.

out[c, :] = concat_e(lattice[idx[c, e], :]) @ W      (c: 262144, e: 9, W: [576, 128])

Sharding: coarse vertices row-split 8 ways; lattice + weight replicated per
core (no collectives). Per core, each 128-vertex tile is gathered with 9
indirect DMAs (one per neighbor; HW indirect DMA gathers one 256B row per
partition), transposed feature-major via the PE, and multiplied against the
weight chunks with PSUM accumulation.
"""
import os
import sys

import numpy as np

sys.path.insert(0, "/opt/trn_rl_repo")

from contextlib import ExitStack

import concourse.bass as bass
import concourse.mybir as mybir
import concourse.tile as tile
from concourse import bacc
from concourse.bass_utils import run_bass_kernel_spmd

P = 128
N_FINE = 1048576
N_COARSE = 262144
VAL = 64
FE = 9
NF = 128
NCORES = 8
ROWS_PER_CORE = N_COARSE // NCORES       # 32768
NT = ROWS_PER_CORE // P                  # 256 tiles per core
KCH = [(0, 128), (128, 128), (256, 128), (384, 128), (512, 64)]

_cached = {}
last_exec_time_ns = None  # set when COARSEN_TRACE=1 and profiling succeeds


def _install_ntff_hook():
    """Register the axon NTFF profile hook (container's antenv lacks axon_hooks)."""
    import contextlib
    import ctypes
    import types

    import antenv

    if getattr(antenv, "axon_hooks", None) is not None:
        return
    state = {}

    def set_hook(h):
        state["h"] = h

    def get_hook():
        return state.get("h")

    mod = types.ModuleType("antenv.axon_hooks")
    mod.set_axon_ntff_profile_hook = set_hook
    mod.get_axon_ntff_profile_hook = get_hook
    sys.modules["antenv.axon_hooks"] = mod
    antenv.axon_hooks = mod

    so_path = "/opt/axon/libaxon_pjrt.so"
    try:
        lib = ctypes.CDLL(so_path)
    except OSError:
        return
    if not hasattr(lib, "axon_start_nrt_profile"):
        return
    lib.axon_start_nrt_profile.argtypes = [ctypes.POINTER(ctypes.c_int64), ctypes.c_size_t]
    lib.axon_start_nrt_profile.restype = ctypes.c_int64
    lib.axon_stop_nrt_profile.argtypes = [ctypes.c_char_p]
    lib.axon_stop_nrt_profile.restype = ctypes.c_int64

    @contextlib.contextmanager
    def _hook_cm(output_dir, device_ids):
        import jax

        jax.devices()
        if device_ids:
            ids = (ctypes.c_int64 * len(device_ids))(*device_ids)
            rc = lib.axon_start_nrt_profile(ids, len(device_ids))
        else:
            rc = lib.axon_start_nrt_profile(None, 0)
        if rc != 0:
            raise RuntimeError(f"axon_start_nrt_profile rc={rc}")
        try:
            yield
        finally:
            n = lib.axon_stop_nrt_profile(str(output_dir).encode())
            if n < 0:
                raise RuntimeError(f"axon_stop_nrt_profile rc={n}")

    set_hook(_hook_cm)


def _build():
    if "nc" in _cached:
        return _cached["nc"]
    nc = bacc.Bacc("TRN2", target_bir_lowering=False, debug=False)
    lattice = nc.dram_tensor("lattice", [N_FINE, VAL], mybir.dt.float32, kind="ExternalInput").ap()
    idx = nc.dram_tensor("idx", [P, NT * FE], mybir.dt.int32, kind="ExternalInput").ap()
    w = nc.dram_tensor("w", [FE * VAL, NF], mybir.dt.float32, kind="ExternalInput").ap()
    ident = nc.dram_tensor("ident", [P, P], mybir.dt.float32, kind="ExternalInput").ap()
    out = nc.dram_tensor("out", [ROWS_PER_CORE, NF], mybir.dt.float32, kind="ExternalOutput").ap()

    with tile.TileContext(nc) as tc, ExitStack() as ctx:
        cpool = ctx.enter_context(tc.tile_pool(name="const", bufs=1))
        rpool = ctx.enter_context(tc.tile_pool(name="r", bufs=12))
        rtpool = ctx.enter_context(tc.tile_pool(name="rt", bufs=6))
        opool = ctx.enter_context(tc.tile_pool(name="o", bufs=4))
        ppool = ctx.enter_context(tc.tile_pool(name="pt", bufs=4, space="PSUM"))
        opsum = ctx.enter_context(tc.tile_pool(name="po", bufs=4, space="PSUM"))

        idx_sb = cpool.tile([P, NT * FE], mybir.dt.int32)
        # tile 0's columns first so the first gather doesn't wait on the
        # full 1.2MB index load; the rest streams in under tile 0's gathers
        nc.sync.dma_start(out=idx_sb[:, 0:FE], in_=idx[:, 0:FE])
        nc.sync.dma_start(out=idx_sb[:, FE:], in_=idx[:, FE:])
        # identity comes from DRAM (host-supplied eye) instead of
        # make_identity, keeping the Pool engine free of preamble work
        identity = cpool.tile([P, P], mybir.dt.float32)
        nc.sync.dma_start(out=identity[:], in_=ident[:])
        w_all = cpool.tile([P, len(KCH) * NF], mybir.dt.float32)
        for k, (k0, kd) in enumerate(KCH):
            nc.sync.dma_start(out=w_all[0:kd, k * NF:(k + 1) * NF], in_=w[k0:k0 + kd, :])

        for t in range(NT):
            r = rpool.tile([P, FE * VAL], mybir.dt.float32)
            for e in range(FE):
                col = t * FE + e
                nc.gpsimd.indirect_dma_start(
                    out=r[:, e * VAL:(e + 1) * VAL],
                    out_offset=None,
                    in_=lattice[:],
                    in_offset=bass.IndirectOffsetOnAxis(ap=idx_sb[:, col:col + 1], axis=0),
                )
            po = opsum.tile([P, NF], mybir.dt.float32)
            for k, (k0, kd) in enumerate(KCH):
                pt = ppool.tile([P, P], mybir.dt.float32)
                nc.tensor.transpose(out=pt[0:kd, :], in_=r[:, k0:k0 + kd], identity=identity[:])
                rt = rtpool.tile([P, P], mybir.dt.float32, tag="rt")
                nc.vector.tensor_copy(out=rt[0:kd, :], in_=pt[0:kd, :])
                nc.tensor.matmul(
                    out=po[:],
                    lhsT=rt[0:kd, :],
                    rhs=w_all[0:kd, k * NF:(k + 1) * NF],
                    start=(k == 0),
                    stop=(k == len(KCH) - 1),
                )
            ot = opool.tile([P, NF], mybir.dt.float32)
            nc.vector.tensor_copy(out=ot[:], in_=po[:])
            nc.sync.dma_start(out=out[t * P:(t + 1) * P, :], in_=ot[:])
    nc.compile()
    _cached["nc"] = nc
    return nc


def _prep_idx(idx_rows):
    """[ROWS_PER_CORE, FE] int -> [P, NT*FE] int32; col t*FE+e holds idx[t*P+p, e]."""
    x = idx_rows.reshape(NT, P, FE).transpose(1, 0, 2).reshape(P, NT * FE)
    return np.ascontiguousarray(x).astype(np.int32)


def kernel(lattice_fine_values, neighbor_indices, weight):
    lattice = np.ascontiguousarray(np.asarray(lattice_fine_values, dtype=np.float32))
    weight = np.ascontiguousarray(np.asarray(weight, dtype=np.float32))
    idx = np.asarray(neighbor_indices)

    nc = _build()
    eye = np.ascontiguousarray(np.eye(P, dtype=np.float32))
    in_maps = []
    for j in range(NCORES):
        shard = idx[j * ROWS_PER_CORE:(j + 1) * ROWS_PER_CORE]
        in_maps.append({"lattice": lattice, "idx": _prep_idx(shard), "w": weight,
                        "ident": eye})
    trace = os.environ.get("COARSEN_TRACE") == "1"
    if trace:
        _install_ntff_hook()
    res = run_bass_kernel_spmd(nc, in_maps, list(range(NCORES)), trace=trace)
    if trace:
        global last_exec_time_ns
        last_exec_time_ns = res.exec_time_ns
    out = np.concatenate([res.results[j]["out"] for j in range(NCORES)], axis=0)
    return out


if __name__ == "__main__":
    rng = np.random.default_rng(0)
    lat = rng.normal(size=(N_FINE, VAL)).astype(np.float32)
    idx = rng.integers(0, N_FINE, size=(N_COARSE, FE)).astype(np.int64)
    w = (rng.normal(size=(FE * VAL, NF)) * 0.05).astype(np.float32)
    out = kernel(lat, idx, w)
    exp = lat[idx].reshape(N_COARSE, FE * VAL) @ w
    err = np.abs(out - exp).max()
    rel = np.abs(out - exp).max() / (np.abs(exp).max() + 1e-9)
    print("max abs err:", err, "rel:", rel)



# revision 5
# speedup vs baseline: 1.0010x; 1.0010x over previous
"""CoarsenLattice forward on 8 Trainium2 NeuronCores.

out[c, :] = concat_e(lattice[idx[c, e], :]) @ W      (c: 262144, e: 9, W: [576, 128])

Sharding: coarse vertices row-split 8 ways; lattice + weight replicated per
core (no collectives). Per core, each 128-vertex tile is gathered with 9
indirect DMAs (one per neighbor; HW indirect DMA gathers one row per
partition). The lattice/weight are cast to bf16 on the host: gather rows are
128B (half the HBM/descriptor traffic) and PE transpose+matmul run at
1 cyc/row instead of 2-4. PSUM accumulates fp32; output stays fp32.
Pool-engine SWDGE descriptor generation (~1us per indirect DMA) is the
hard bottleneck; deep gather-buffer pipelining keeps Pool issuing
back-to-back while PE/DVE/ACT trail behind.
"""
import os
import sys

import numpy as np

sys.path.insert(0, "/opt/trn_rl_repo")

from contextlib import ExitStack

import ml_dtypes

import concourse.bass as bass
import concourse.mybir as mybir
import concourse.tile as tile
from concourse import bacc
from concourse.bass_utils import run_bass_kernel_spmd

P = 128
N_FINE = 1048576
N_COARSE = 262144
VAL = 64
FE = 9
NF = 128
NCORES = 8
ROWS_PER_CORE = N_COARSE // NCORES       # 32768
NT = ROWS_PER_CORE // P                  # 256 tiles per core
KCH = [(0, 128), (128, 128), (256, 128), (384, 128), (512, 64)]
BF16 = mybir.dt.bfloat16

_cached = {}
last_exec_time_ns = None  # set when COARSEN_TRACE=1 and profiling succeeds


def _install_ntff_hook():
    """Register the axon NTFF profile hook (container's antenv lacks axon_hooks)."""
    import contextlib
    import ctypes
    import types

    import antenv

    if getattr(antenv, "axon_hooks", None) is not None:
        return
    state = {}

    def set_hook(h):
        state["h"] = h

    def get_hook():
        return state.get("h")

    mod = types.ModuleType("antenv.axon_hooks")
    mod.set_axon_ntff_profile_hook = set_hook
    mod.get_axon_ntff_profile_hook = get_hook
    sys.modules["antenv.axon_hooks"] = mod
    antenv.axon_hooks = mod

    so_path = "/opt/axon/libaxon_pjrt.so"
    try:
        lib = ctypes.CDLL(so_path)
    except OSError:
        return
    if not hasattr(lib, "axon_start_nrt_profile"):
        return
    lib.axon_start_nrt_profile.argtypes = [ctypes.POINTER(ctypes.c_int64), ctypes.c_size_t]
    lib.axon_start_nrt_profile.restype = ctypes.c_int64
    lib.axon_stop_nrt_profile.argtypes = [ctypes.c_char_p]
    lib.axon_stop_nrt_profile.restype = ctypes.c_int64

    @contextlib.contextmanager
    def _hook_cm(output_dir, device_ids):
        import jax

        jax.devices()
        if device_ids:
            ids = (ctypes.c_int64 * len(device_ids))(*device_ids)
            rc = lib.axon_start_nrt_profile(ids, len(device_ids))
        else:
            rc = lib.axon_start_nrt_profile(None, 0)
        if rc != 0:
            raise RuntimeError(f"axon_start_nrt_profile rc={rc}")
        try:
            yield
        finally:
            n = lib.axon_stop_nrt_profile(str(output_dir).encode())
            if n < 0:
                raise RuntimeError(f"axon_stop_nrt_profile rc={n}")

    set_hook(_hook_cm)


def _build():
    if "nc" in _cached:
        return _cached["nc"]
    nc = bacc.Bacc("TRN2", target_bir_lowering=False, debug=False)
    lattice = nc.dram_tensor("lattice", [N_FINE, VAL], BF16, kind="ExternalInput").ap()
    idx = nc.dram_tensor("idx", [P, NT * FE], mybir.dt.int32, kind="ExternalInput").ap()
    w = nc.dram_tensor("w", [FE * VAL, NF], BF16, kind="ExternalInput").ap()
    ident = nc.dram_tensor("ident", [P, P], BF16, kind="ExternalInput").ap()
    out = nc.dram_tensor("out", [ROWS_PER_CORE, NF], mybir.dt.float32, kind="ExternalOutput").ap()

    with tile.TileContext(nc) as tc, ExitStack() as ctx:
        cpool = ctx.enter_context(tc.tile_pool(name="const", bufs=1))
        # deep gather-destination pool: Pool engine must never stall on a
        # free buffer (bf16 tile = 1152B/partition; 48 bufs = 54KB/partition)
        rpool = ctx.enter_context(tc.tile_pool(name="r", bufs=48))
        rtpool = ctx.enter_context(tc.tile_pool(name="rt", bufs=8))
        opool = ctx.enter_context(tc.tile_pool(name="o", bufs=6))
        ppool = ctx.enter_context(tc.tile_pool(name="pt", bufs=4, space="PSUM"))
        opsum = ctx.enter_context(tc.tile_pool(name="po", bufs=4, space="PSUM"))

        idx_sb = cpool.tile([P, NT * FE], mybir.dt.int32)
        # tile 0's columns first so the first gather doesn't wait on the
        # full 1.2MB index load; the rest streams in under tile 0's gathers
        nc.sync.dma_start(out=idx_sb[:, 0:FE], in_=idx[:, 0:FE])
        nc.sync.dma_start(out=idx_sb[:, FE:], in_=idx[:, FE:])
        identity = cpool.tile([P, P], BF16)
        nc.sync.dma_start(out=identity[:], in_=ident[:])
        w_all = cpool.tile([P, len(KCH) * NF], BF16)
        for k, (k0, kd) in enumerate(KCH):
            nc.sync.dma_start(out=w_all[0:kd, k * NF:(k + 1) * NF], in_=w[k0:k0 + kd, :])

        for t in range(NT):
            r = rpool.tile([P, FE * VAL], BF16)
            for e in range(FE):
                col = t * FE + e
                nc.gpsimd.indirect_dma_start(
                    out=r[:, e * VAL:(e + 1) * VAL],
                    out_offset=None,
                    in_=lattice[:],
                    in_offset=bass.IndirectOffsetOnAxis(ap=idx_sb[:, col:col + 1], axis=0),
                )
            po = opsum.tile([P, NF], mybir.dt.float32)
            for k, (k0, kd) in enumerate(KCH):
                pt = ppool.tile([P, P], BF16)
                nc.tensor.transpose(out=pt[0:kd, :], in_=r[:, k0:k0 + kd], identity=identity[:])
                rt = rtpool.tile([P, P], BF16, tag="rt")
                # alternate PSUM->SBUF chunk copies between DVE and ACT so
                # neither engine trails the PE
                if k % 2 == 0:
                    nc.vector.tensor_copy(out=rt[0:kd, :], in_=pt[0:kd, :])
                else:
                    nc.scalar.copy(out=rt[0:kd, :], in_=pt[0:kd, :])
                nc.tensor.matmul(
                    out=po[:],
                    lhsT=rt[0:kd, :],
                    rhs=w_all[0:kd, k * NF:(k + 1) * NF],
                    start=(k == 0),
                    stop=(k == len(KCH) - 1),
                )
            ot = opool.tile([P, NF], mybir.dt.float32)
            if t % 2 == 0:
                nc.vector.tensor_copy(out=ot[:], in_=po[:])
            else:
                nc.scalar.copy(out=ot[:], in_=po[:])
            nc.sync.dma_start(out=out[t * P:(t + 1) * P, :], in_=ot[:])
    nc.compile()
    _cached["nc"] = nc
    return nc


def _prep_idx(idx_rows):
    """[ROWS_PER_CORE, FE] int -> [P, NT*FE] int32; col t*FE+e holds idx[t*P+p, e]."""
    x = idx_rows.reshape(NT, P, FE).transpose(1, 0, 2).reshape(P, NT * FE)
    return np.ascontiguousarray(x).astype(np.int32)


def kernel(lattice_fine_values, neighbor_indices, weight):
    lattice = np.ascontiguousarray(
        np.asarray(lattice_fine_values, dtype=np.float32).astype(ml_dtypes.bfloat16)
    )
    weight = np.ascontiguousarray(
        np.asarray(weight, dtype=np.float32).astype(ml_dtypes.bfloat16)
    )
    idx = np.asarray(neighbor_indices)

    nc = _build()
    eye = np.ascontiguousarray(np.eye(P, dtype=np.float32).astype(ml_dtypes.bfloat16))
    in_maps = []
    for j in range(NCORES):
        shard = idx[j * ROWS_PER_CORE:(j + 1) * ROWS_PER_CORE]
        in_maps.append({"lattice": lattice, "idx": _prep_idx(shard), "w": weight,
                        "ident": eye})
    trace = os.environ.get("COARSEN_TRACE") == "1"
    if trace:
        _install_ntff_hook()
    res = run_bass_kernel_spmd(nc, in_maps, list(range(NCORES)), trace=trace)
    if trace:
        global last_exec_time_ns
        last_exec_time_ns = res.exec_time_ns
    out = np.concatenate([np.asarray(res.results[j]["out"]) for j in range(NCORES)], axis=0)
    return out


if __name__ == "__main__":
    rng = np.random.default_rng(0)
    lat = rng.normal(size=(N_FINE, VAL)).astype(np.float32)
    idx = rng.integers(0, N_FINE, size=(N_COARSE, FE)).astype(np.int64)
    w = (rng.normal(size=(FE * VAL, NF)) * 0.05).astype(np.float32)
    out = kernel(lat, idx, w)
    exp = lat[idx].reshape(N_COARSE, FE * VAL) @ w
    err = np.abs(out - exp).max()
    rel = np.abs(out - exp).max() / (np.abs(exp).max() + 1e-9)
    print("max abs err:", err, "rel:", rel)


# revision 7
# speedup vs baseline: 1.0016x; 1.0006x over previous
"""CoarsenLattice forward on 8 Trainium2 NeuronCores.

out[c, :] = concat_e(lattice[idx[c, e], :]) @ W      (c: 262144, e: 9, W: [576, 128])

Sharding: coarse vertices row-split 8 ways; lattice + weight replicated per
core (no collectives). Per core, each 128-vertex tile is gathered with 9
indirect DMAs (one per neighbor; HW indirect DMA gathers one row per
partition). The lattice/weight are cast to bf16 on the host: gather rows are
128B (half the HBM/descriptor traffic) and PE transpose+matmul run at
1 cyc/row instead of 2-4. PSUM accumulates fp32; output stays fp32.
Pool-engine SWDGE descriptor generation (~1us per indirect DMA) is the
hard bottleneck; deep gather-buffer pipelining keeps Pool issuing
back-to-back while PE/DVE/ACT trail behind.
"""
import os
import sys

import numpy as np

sys.path.insert(0, "/opt/trn_rl_repo")

from contextlib import ExitStack

import ml_dtypes

import concourse.bass as bass
import concourse.mybir as mybir
import concourse.tile as tile
from concourse import bacc
from concourse.bass_utils import run_bass_kernel_spmd

P = 128
N_FINE = 1048576
N_COARSE = 262144
VAL = 64
FE = 9
NF = 128
NCORES = 8
ROWS_PER_CORE = N_COARSE // NCORES       # 32768
NT = ROWS_PER_CORE // P                  # 256 tiles per core
KCH = [(0, 128), (128, 128), (256, 128), (384, 128), (512, 64)]
BF16 = mybir.dt.bfloat16

_cached = {}
last_exec_time_ns = None  # set when COARSEN_TRACE=1 and profiling succeeds


def _install_ntff_hook():
    """Register the axon NTFF profile hook (container's antenv lacks axon_hooks)."""
    import contextlib
    import ctypes
    import types

    import antenv

    if getattr(antenv, "axon_hooks", None) is not None:
        return
    state = {}

    def set_hook(h):
        state["h"] = h

    def get_hook():
        return state.get("h")

    mod = types.ModuleType("antenv.axon_hooks")
    mod.set_axon_ntff_profile_hook = set_hook
    mod.get_axon_ntff_profile_hook = get_hook
    sys.modules["antenv.axon_hooks"] = mod
    antenv.axon_hooks = mod

    so_path = "/opt/axon/libaxon_pjrt.so"
    try:
        lib = ctypes.CDLL(so_path)
    except OSError:
        return
    if not hasattr(lib, "axon_start_nrt_profile"):
        return
    lib.axon_start_nrt_profile.argtypes = [ctypes.POINTER(ctypes.c_int64), ctypes.c_size_t]
    lib.axon_start_nrt_profile.restype = ctypes.c_int64
    lib.axon_stop_nrt_profile.argtypes = [ctypes.c_char_p]
    lib.axon_stop_nrt_profile.restype = ctypes.c_int64

    @contextlib.contextmanager
    def _hook_cm(output_dir, device_ids):
        import jax

        jax.devices()
        if device_ids:
            ids = (ctypes.c_int64 * len(device_ids))(*device_ids)
            rc = lib.axon_start_nrt_profile(ids, len(device_ids))
        else:
            rc = lib.axon_start_nrt_profile(None, 0)
        if rc != 0:
            raise RuntimeError(f"axon_start_nrt_profile rc={rc}")
        try:
            yield
        finally:
            n = lib.axon_stop_nrt_profile(str(output_dir).encode())
            if n < 0:
                raise RuntimeError(f"axon_stop_nrt_profile rc={n}")

    set_hook(_hook_cm)


def _build():
    if "nc" in _cached:
        return _cached["nc"]
    nc = bacc.Bacc("TRN2", target_bir_lowering=False, debug=False)
    lattice = nc.dram_tensor("lattice", [N_FINE, VAL], BF16, kind="ExternalInput").ap()
    idx = nc.dram_tensor("idx", [P, NT * FE], mybir.dt.int32, kind="ExternalInput").ap()
    w = nc.dram_tensor("w", [FE * VAL, NF], BF16, kind="ExternalInput").ap()
    ident = nc.dram_tensor("ident", [P, P], BF16, kind="ExternalInput").ap()
    out = nc.dram_tensor("out", [ROWS_PER_CORE, NF], mybir.dt.float32, kind="ExternalOutput").ap()

    with tile.TileContext(nc) as tc, ExitStack() as ctx:
        cpool = ctx.enter_context(tc.tile_pool(name="const", bufs=1))
        # deep gather-destination pool: Pool engine must never stall on a
        # free buffer (bf16 tile = 1152B/partition; 48 bufs = 54KB/partition)
        rpool = ctx.enter_context(tc.tile_pool(name="r", bufs=48))
        rtpool = ctx.enter_context(tc.tile_pool(name="rt", bufs=8))
        opool = ctx.enter_context(tc.tile_pool(name="o", bufs=6))
        ppool = ctx.enter_context(tc.tile_pool(name="pt", bufs=4, space="PSUM"))
        opsum = ctx.enter_context(tc.tile_pool(name="po", bufs=4, space="PSUM"))

        idx_sb = cpool.tile([P, NT * FE], mybir.dt.int32)
        # tile 0's columns first so the first gather doesn't wait on the
        # full 1.2MB index load; the rest streams in under tile 0's gathers
        nc.sync.dma_start(out=idx_sb[:, 0:FE], in_=idx[:, 0:FE])
        nc.sync.dma_start(out=idx_sb[:, FE:], in_=idx[:, FE:])
        identity = cpool.tile([P, P], BF16)
        nc.sync.dma_start(out=identity[:], in_=ident[:])
        w_all = cpool.tile([P, len(KCH) * NF], BF16)
        for k, (k0, kd) in enumerate(KCH):
            nc.sync.dma_start(out=w_all[0:kd, k * NF:(k + 1) * NF], in_=w[k0:k0 + kd, :])

        for t in range(NT):
            r = rpool.tile([P, FE * VAL], BF16)
            for e in range(FE):
                col = t * FE + e
                nc.gpsimd.indirect_dma_start(
                    out=r[:, e * VAL:(e + 1) * VAL],
                    out_offset=None,
                    in_=lattice[:],
                    in_offset=bass.IndirectOffsetOnAxis(ap=idx_sb[:, col:col + 1], axis=0),
                )
            po = opsum.tile([P, NF], mybir.dt.float32)
            for k, (k0, kd) in enumerate(KCH):
                pt = ppool.tile([P, P], BF16)
                nc.tensor.transpose(out=pt[0:kd, :], in_=r[:, k0:k0 + kd], identity=identity[:])
                rt = rtpool.tile([P, P], BF16, tag="rt")
                # alternate PSUM->SBUF chunk copies between DVE and ACT so
                # neither engine trails the PE
                if k % 2 == 0:
                    nc.vector.tensor_copy(out=rt[0:kd, :], in_=pt[0:kd, :])
                else:
                    nc.scalar.copy(out=rt[0:kd, :], in_=pt[0:kd, :])
                nc.tensor.matmul(
                    out=po[:],
                    lhsT=rt[0:kd, :],
                    rhs=w_all[0:kd, k * NF:(k + 1) * NF],
                    start=(k == 0),
                    stop=(k == len(KCH) - 1),
                )
            ot = opool.tile([P, NF], mybir.dt.float32)
            if t % 2 == 0:
                nc.vector.tensor_copy(out=ot[:], in_=po[:])
            else:
                nc.scalar.copy(out=ot[:], in_=po[:])
            nc.sync.dma_start(out=out[t * P:(t + 1) * P, :], in_=ot[:])
    nc.compile()
    _cached["nc"] = nc
    return nc


def _prep_idx(idx_rows):
    """[ROWS_PER_CORE, FE] int -> [P, NT*FE] int32; col t*FE+e holds idx[t*P+p, e]."""
    x = idx_rows.reshape(NT, P, FE).transpose(1, 0, 2).reshape(P, NT * FE)
    return np.ascontiguousarray(x).astype(np.int32)


def kernel(lattice_fine_values, neighbor_indices, weight):
    lattice = np.ascontiguousarray(
        np.asarray(lattice_fine_values, dtype=np.float32).astype(ml_dtypes.bfloat16)
    )
    weight = np.ascontiguousarray(
        np.asarray(weight, dtype=np.float32).astype(ml_dtypes.bfloat16)
    )
    idx = np.asarray(neighbor_indices)

    nc = _build()
    eye = np.ascontiguousarray(np.eye(P, dtype=np.float32).astype(ml_dtypes.bfloat16))
    in_maps = []
    for j in range(NCORES):
        shard = idx[j * ROWS_PER_CORE:(j + 1) * ROWS_PER_CORE]
        in_maps.append({"lattice": lattice, "idx": _prep_idx(shard), "w": weight,
                        "ident": eye})
    trace = os.environ.get("COARSEN_TRACE") == "1"
    if trace:
        _install_ntff_hook()
    res = run_bass_kernel_spmd(nc, in_maps, list(range(NCORES)), trace=trace)
    if trace:
        global last_exec_time_ns
        last_exec_time_ns = res.exec_time_ns
    out = np.concatenate([np.asarray(res.results[j]["out"]) for j in range(NCORES)], axis=0)
    return out


if __name__ == "__main__":
    rng = np.random.default_rng(0)
    lat = rng.normal(size=(N_FINE, VAL)).astype(np.float32)
    idx = rng.integers(0, N_FINE, size=(N_COARSE, FE)).astype(np.int64)
    w = (rng.normal(size=(FE * VAL, NF)) * 0.05).astype(np.float32)
    out = kernel(lat, idx, w)
    exp = lat[idx].reshape(N_COARSE, FE * VAL) @ w
    err = np.abs(out - exp).max()
    rel = np.abs(out - exp).max() / (np.abs(exp).max() + 1e-9)
    print("max abs err:", err, "rel:", rel)
